# revision 34
# baseline (speedup 1.0000x reference)
"""Causal self-attention (D=1024, H=16, S=2048, B=2) on 8 trn2 cores.

Sharding: core i handles batch b = i // 4 and head-group g = i % 4
(4 heads = 256 model dims per group). Each core computes
    y_partial[b,g] = softmax_causal(Q K^T / 8) V  @ Wo[rows of g]
for its 4 heads; the host sums the 4 group partials per batch and adds bo.

Per-core kernel (bf16 matmul operands, fp32 PSUM accumulation), pipelined
over 512-query s-blocks so projections, attention and the output
projection overlap across blocks:
  phase 0: xT[c] <- DMA-transpose of x columns (bf16 XBAR path)
  per s-block sb:
    p1: QT/KT[dc][sb] = (Wq/Wk)^T x^T + b (head pairs packed on
        partitions; bias added on ACT), V tiles for sb's 4 t-chunks
        (+ ones column for the softmax denominator trick)
    p2: per (head-pair dc): scoresT = KT^T QT with causal block skipping,
        exp on ACT (2-plane tiles), triangular diag-chunk mask via gpsimd
        affine_select, PV accumulation -> [65, s] whose row 64 is the
        denominator; reciprocal + DRAM-bounce broadcast + DVE multiply ->
        normalized A^T packed into head-pair tiles AT[dc][sb]
    p3: y[sb] = A Wo with K=128 head-pair accumulation (psum->sbuf on DVE)
"""

import sys

sys.path.insert(0, "/opt/trn_rl_repo")

import ml_dtypes
import numpy as np

import concourse.bass as bass
import concourse.mybir as mybir
import concourse.tile as tile
from concourse import bacc

P = 128
S = 2048
D = 1024
NH = 4                    # heads per core
DH = 64                   # head dim
DPC = NH * DH             # model dims per core = 256
N_CT = D // P             # 8 contraction chunks
N_ST = S // P             # 16 t tiles of 128
N_SB = S // 512           # 4 s blocks of 512
F32 = mybir.dt.float32
BF16 = mybir.dt.bfloat16
SCALE = 1.0 / 8.0         # 1/sqrt(64)

AF = mybir.ActivationFunctionType
ALU = mybir.AluOpType


def build_nc(mm_mode: str = "bf16", stop_after: int = 99,
             skip_norm: bool = False, n_rep: int = 1) -> bass.Bass:
    nc = _build(mm_mode, n_rep)
    if not nc.is_finalized():
        nc.finalize()
    return nc


def _build(mm_mode: str, n_rep: int) -> bass.Bass:
    assert mm_mode == "bf16"
    nc = bacc.Bacc("TRN2", target_bir_lowering=False, debug=False,
                   num_devices=8)

    x_d = nc.dram_tensor("x", [S, D], BF16, kind="ExternalInput")
    wq_d = nc.dram_tensor("wq", [D, DPC], BF16, kind="ExternalInput")
    wk_d = nc.dram_tensor("wk", [D, DPC], BF16, kind="ExternalInput")
    wv_d = nc.dram_tensor("wv", [D, DPC], BF16, kind="ExternalInput")
    wo_d = nc.dram_tensor("wo", [DPC, D], BF16, kind="ExternalInput")
    bq_d = nc.dram_tensor("bq", [DPC], F32, kind="ExternalInput")
    bk_d = nc.dram_tensor("bk", [DPC], F32, kind="ExternalInput")
    bv_d = nc.dram_tensor("bv", [DPC], F32, kind="ExternalInput")
    y_d = nc.dram_tensor("y", [S, D], F32, kind="ExternalOutput")

    with tile.TileContext(nc) as tc:
        with (
            tc.tile_pool(name="const", bufs=1) as const,
            tc.tile_pool(name="xtp", bufs=1) as xtp,
            tc.tile_pool(name="qkv", bufs=1) as qkv,
            tc.tile_pool(name="atp", bufs=1) as atp,
            tc.tile_pool(name="work", bufs=4) as work,
            tc.tile_pool(name="att", bufs=4) as attw,
            tc.tile_pool(name="denp", bufs=4) as denp,
            tc.tile_pool(name="ps", bufs=3, space="PSUM") as psp,
            tc.tile_pool(name="ppv", bufs=2, space="PSUM") as ppv,
        ):
            # ---- HAM warm-up: keep PE busy during the DMA head so the
            # real matmuls start at 2.4 GHz (clock gate released) ----
            wrm = work.tile([P, 512], BF16, tag="wrm")
            nc.vector.memset(wrm, 0.0)
            wps = psp.tile([P, 2, 512], F32, tag="ps")
            for i in range(28):
                nc.tensor.matmul(wps[:, 0, :], wrm[:, 0:P], wrm,
                                 start=True, stop=True)
            ones_s = const.tile([1, DH], BF16)
            nc.vector.memset(ones_s, 1.0)

            # ---- weights / constants (loaded once, ordered by first use) --
            wq_s = const.tile([P, N_CT, DPC], BF16)
            wk_s = const.tile([P, N_CT, DPC], BF16)
            wv_s = const.tile([P, N_CT, DPC], BF16)
            # Wo packed by head pairs: rows 128*dc .. 128*dc+127
            wo_s = const.tile([P, 2, D], BF16)
            nc.sync.dma_start(wq_s, wq_d.rearrange("(o p) d -> p o d", p=P))
            nc.sync.dma_start(wk_s, wk_d.rearrange("(o p) d -> p o d", p=P))
            nc.sync.dma_start(wv_s, wv_d.rearrange("(o p) d -> p o d", p=P))
            nc.sync.dma_start(wo_s, wo_d.rearrange("(dc p) e -> p dc e", p=P))
            bq_s = const.tile([P, 2], F32)
            bk_s = const.tile([P, 2], F32)
            nc.sync.dma_start(bq_s, bq_d.rearrange("(o p) -> p o", p=P))
            nc.sync.dma_start(bk_s, bk_d.rearrange("(o p) -> p o", p=P))
            bv_b = const.tile([P, DPC], F32)
            nc.gpsimd.dma_start(
                out=bv_b, in_=bv_d[:].unsqueeze(0).partition_broadcast(P)
            )

            weights = (wq_s, wk_s, wv_s, wo_s, bq_s, bk_s, bv_b)
            emit_iteration = _make_iteration(nc, tc, const, xtp, qkv, atp,
                                             work, attw, denp, psp, ppv,
                                             ones_s, weights, x_d, wv_d,
                                             wo_d, y_d)
            for rep in range(n_rep):
                emit_iteration(f"r{rep}", rep % 2)

    return nc


def _make_iteration(nc, tc, const, xtp, qkv, atp, work, attw, denp, psp,
                    ppv, ones_s, weights, x_d, wv_d, wo_d, y_d):
    wq_s, wk_s, wv_s, wo_s, bq_s, bk_s, bv_b = weights

    def emit_iteration(sfx, par=0):
            # ---- phase 0: DMA-transpose x into per-chunk xT tiles ----
            # s-block-major so phase-1 groups for early s-blocks can start
            # while later transposes are still on the XBAR. wv/wo loads are
            # interleaved after the transposes their consumers wait behind.
            xT = [xtp.tile([P, S], BF16, tag=f"xt{c}p{par}", name=f"xt{c}{sfx}")
                  for c in range(N_CT)]
            for g in range(N_SB):
                for c in range(N_CT):
                    nc.sync.dma_start_transpose(
                        xT[c][:, g * 512:(g + 1) * 512],
                        x_d[g * 512:(g + 1) * 512, c * P:(c + 1) * P])

            # ---- persistent per-block tiles (fine-grained deps) ----
            # QT/KT: [128 (head-pair d), s] per (dc, sb)
            QTs = [[qkv.tile([P, 512], BF16, tag=f"qt{dc}_{sb}",
                             name=f"qt{dc}_{sb}{sfx}") for sb in range(N_SB)]
                   for dc in range(2)]
            KTs = [[qkv.tile([P, 512], BF16, tag=f"kt{dc}_{sb}",
                             name=f"kt{dc}_{sb}{sfx}") for sb in range(N_SB)]
                   for dc in range(2)]
            # V_aug per t-chunk: [t-part, head, 65], col 64 == 1.0
            vaugs = [qkv.tile([P, NH, DH + 1], BF16, tag=f"va{tt}",
                              name=f"va{tt}{sfx}") for tt in range(N_ST)]
            # normalized A^T packed by head pairs: [128, 512] per (dc, sb)
            ATs = [[atp.tile([P, 512], BF16, tag=f"at{dc}_{sb}",
                             name=f"at{dc}_{sb}{sfx}") for sb in range(N_SB)]
                   for dc in range(2)]

            # ---- emission helpers (software pipelining) ----
            # Per-engine execution is in-order, so program order IS the
            # schedule. Phase-1 groups for block sb+1 and phase-3 stripes
            # for block sb-1 are emitted as "fillers" between attention
            # T-iterations, filling PE bubbles while ACT streams exp.

            # Filler units are (matmul-part, eviction-part) pairs queued
            # separately so the in-order ACT/DVE streams never reach an
            # eviction before its matmul group has finished.

            def emit_qk(dc, sb):
                """Q and K projections for (dc, sb): one 2-plane psum slot;
                eviction (deferred) applies the per-partition biases on ACT."""
                ps = psp.tile([P, 2, 512], F32, tag="ps")
                for i, w_s in enumerate((wq_s, wk_s)):
                    for c in range(N_CT):
                        nc.tensor.matmul(
                            ps[:, i, :],
                            w_s[:, c, dc * P:(dc + 1) * P],
                            xT[c][:, sb * 512:(sb + 1) * 512],
                            start=(c == 0),
                            stop=(c == N_CT - 1),
                        )

                def evict():
                    nc.scalar.activation(
                        QTs[dc][sb], ps[:, 0, :], AF.Identity,
                        bias=bq_s[:, dc:dc + 1],
                    )
                    nc.scalar.activation(
                        KTs[dc][sb], ps[:, 1, :], AF.Identity,
                        bias=bk_s[:, dc:dc + 1],
                    )
                return evict

            def emit_v(sb, ti):
                """V projections for t-tiles (4sb+2ti, 4sb+2ti+1): one
                2-plane psum slot; deferred DVE bias adds."""
                ps = psp.tile([P, 2, 512], F32, tag="ps")
                for i in range(2):
                    tt = 4 * sb + 2 * ti + i
                    pvs = ps[:, i, 0:DPC]
                    for c in range(N_CT):
                        nc.tensor.matmul(
                            pvs,
                            xT[c][:, tt * P:(tt + 1) * P],
                            wv_s[:, c, :],
                            start=(c == 0),
                            stop=(c == N_CT - 1),
                        )

                def evict():
                    for i in range(2):
                        tt = 4 * sb + 2 * ti + i
                        nc.vector.memset(vaugs[tt][:, :, DH:DH + 1], 1.0)
                        nc.vector.tensor_add(
                            vaugs[tt][:, :, 0:DH],
                            ps[:, i, 0:DPC].rearrange("p (h u) -> p h u", h=NH),
                            bv_b.rearrange("p (h u) -> p h u", h=NH),
                        )
                return evict

            def emit_p3(sb, stl):
                """Output-projection stripe: both half-blocks of one 128-row
                stripe share a 2-plane psum slot; deferred DVE eviction."""
                ps = psp.tile([P, 2, 512], F32, tag="ps")
                for eb in range(2):
                    for dc in range(2):
                        nc.tensor.matmul(
                            ps[:, eb, :],
                            ATs[dc][sb][:, stl * P:(stl + 1) * P],
                            wo_s[:, dc, eb * 512:(eb + 1) * 512],
                            start=(dc == 0),
                            stop=(dc == 1),
                        )

                def evict():
                    ys = work.tile([P, D], F32, tag="work")
                    nc.vector.tensor_copy(
                        ys, ps.rearrange("p a b -> p (a b)"))
                    st = 4 * sb + stl
                    nc.sync.dma_start(y_d[st * P:(st + 1) * P, :], ys)
                return evict

            def emit_normalize_pair(dc, sb, pvs2):
                """Evict both PV psums to SBUF first (frees the ppv slots
                fast), then reciprocal + PE ones-matmul broadcast + DVE
                multiply per head."""
                # e=1 evicts on ACT (free right after the last exp), e=0 on
                # DVE — both PV slots free in parallel, recips start sooner
                pvcs = [denp.tile([DH + 1, 512], F32, tag=f"pvc{e}",
                                  name=f"pvc{e}{sfx}")
                        for e in range(2)]
                nc.scalar.copy(pvcs[1], pvs2[1])
                nc.vector.tensor_copy(pvcs[0], pvs2[0])
                for e in (1, 0):  # e=1 first: its extra AT DMA gains latency
                    pvc = pvcs[e]
                    rden = denp.tile([1, 512], BF16, tag="rden",
                                     name=f"rden{sfx}")
                    with nc.allow_low_precision(
                            reason="softmax denom recip, 2e-2 tol"):
                        nc.vector.reciprocal(out=rden, in_=pvc[DH:DH + 1, :])
                    # broadcast recip down 64 partitions: [1,DH].T @ [1,512]
                    rb = ppv.tile([DH, 512], F32, tag="pv",
                                  name=f"rb{dc}{sb}{e}{sfx}")
                    nc.tensor.matmul(rb, ones_s, rden, start=True, stop=True)
                    if e == 0:
                        nc.vector.tensor_mul(
                            ATs[dc][sb][0:DH, :], pvc[0:DH, :], rb)
                    else:
                        att = attw.tile([DH, 512], BF16, tag="att")
                        nc.vector.tensor_mul(att, pvc[0:DH, :], rb)
                        nc.sync.dma_start(ATs[dc][sb][DH:P, :], att)

            # filler units, queued in dependency-deadline order; a unit is
            # (fn, pace): fn may return an eviction closure, re-queued as
            # the next unit so it runs ~pace T-iterations after the matmuls
            from collections import deque
            fillers = deque()

            def pop_filler(T, next_ok):
                fn, pace = fillers.popleft()
                nxt = fn()
                if nxt is not None:
                    fillers.appendleft((nxt, 1))
                next_ok[0] = T + pace

            # head: everything phase 2 of (dc=0, sb=0) needs, plus the
            # next block's first QK group (the DMA head has PE slack)
            emit_qk(0, 0)()
            emit_v(0, 0)()
            emit_v(0, 1)()
            emit_qk(0, 1)()

            for sb in range(N_SB):
                t_cnt = 4 * sb + 4
                for dc in range(2):
                    if dc == 0:
                        # queue next block's phase 1 at each period start;
                        # phase-3 stripes go late (sb2: block 0; sb3:
                        # blocks 1+2) where the long exp chains leave the
                        # most PE slack
                        if sb == 0:
                            fillers.append((lambda: emit_qk(1, 0), 2))
                            fillers.append((lambda: emit_v(1, 0), 2))
                            fillers.append((lambda: emit_v(1, 1), 2))
                            fillers.append((lambda: emit_qk(1, 1), 2))
                        elif sb + 1 < N_SB:
                            fillers.append(
                                (lambda s=sb + 1: emit_qk(0, s), 2))
                            fillers.append(
                                (lambda s=sb + 1: emit_v(s, 0), 2))
                            fillers.append(
                                (lambda s=sb + 1: emit_v(s, 1), 2))
                            fillers.append(
                                (lambda s=sb + 1: emit_qk(1, s), 2))
                        for s in ([0] if sb == 2 else [1, 2] if sb == 3
                                  else []):
                            for stl in range(4):
                                fillers.append(
                                    (lambda s=s, i=stl: emit_p3(s, i), 2))
                    pvs2 = [ppv.tile([DH + 1, 512], F32, tag="pv",
                                     name=f"pv{dc}_{sb}_{e}{sfx}")
                            for e in range(2)]
                    next_ok = [1]
                    for T in range(t_cnt):
                        k = T - 4 * sb
                        ms = 128 * k if k > 0 else 0
                        sc = psp.tile([P, 2, 512], F32, tag="ps")
                        ex = attw.tile([P, 2, 512], BF16, tag="ex")
                        for e in range(2):  # even/odd head of the pair
                            off = DH * e
                            nc.tensor.matmul(
                                sc[:, e, ms:512],
                                KTs[dc][T // 4][off:off + DH,
                                                (T % 4) * P:(T % 4 + 1) * P],
                                QTs[dc][sb][off:off + DH, ms:512],
                                start=True,
                                stop=True,
                            )
                        nc.scalar.activation(
                            ex[:, :, ms:512], sc[:, :, ms:512],
                            AF.Exp, scale=SCALE,
                        )
                        if k >= 0:  # triangular mask on diagonal chunks
                            nc.gpsimd.affine_select(
                                out=ex[:, :, ms:ms + P],
                                in_=ex[:, :, ms:ms + P],
                                compare_op=ALU.is_ge,
                                fill=0.0,
                                base=0,
                                pattern=[[0, 2], [1, P]],
                                channel_multiplier=-1,
                            )
                        for e in range(2):
                            h = 2 * dc + e
                            nc.tensor.matmul(
                                pvs2[e][:, ms:512],
                                vaugs[T][:, h, :],
                                ex[:, e, ms:512],
                                start=(T == 0),
                                stop=(T == t_cnt - 1),
                            )
                        if fillers and T >= next_ok[0]:
                            pop_filler(T, next_ok)
                    emit_normalize_pair(dc, sb, pvs2)
                # tail: drain the queue, then the last block's phase 3
                if sb == N_SB - 1:
                    while fillers:
                        fn, _ = fillers.popleft()
                        nxt = fn()
                        if nxt is not None:
                            fillers.appendleft((nxt, 1))
                    for stl in range(4):
                        emit_p3(sb, stl)()

    return emit_iteration


_NC_CACHE = {}


def _get_nc(mm_mode="bf16", n_rep=1):
    key = (mm_mode, n_rep)
    if key not in _NC_CACHE:
        _NC_CACHE[key] = build_nc(mm_mode=mm_mode, n_rep=n_rep)
    return _NC_CACHE[key]


MM_MODE = "bf16"


def make_in_maps(x, Wq, bq, Wk, bk, Wv, bv, Wo, mm_mode=None):
    """Per-core input dicts: core i -> (batch i//4, head-group i%4)."""
    bf = ml_dtypes.bfloat16
    in_maps = []
    for core in range(8):
        b, g = core // 4, core % 4
        sl = slice(g * DPC, (g + 1) * DPC)
        in_maps.append({
            "x": np.ascontiguousarray(x[b]).astype(bf),
            "wq": np.ascontiguousarray(Wq[:, sl]).astype(bf),
            "wk": np.ascontiguousarray(Wk[:, sl]).astype(bf),
            "wv": np.ascontiguousarray(Wv[:, sl]).astype(bf),
            "wo": np.ascontiguousarray(Wo[sl, :]).astype(bf),
            "bq": np.ascontiguousarray(bq[sl]).astype(np.float32),
            "bk": np.ascontiguousarray(bk[sl]).astype(np.float32),
            "bv": np.ascontiguousarray(bv[sl]).astype(np.float32),
        })
    return in_maps


def combine_results(results, bo):
    out = np.zeros((2, S, D), dtype=np.float32)
    for core in range(8):
        out[core // 4] += results[core]["y"]
    out += bo.astype(np.float32)
    return out


_RUNNER_CACHE = {}


def get_runner(mm_mode=None, n_rep=1):
    """Build (once) a jitted 8-core runner; returns fn(in_maps) -> results."""
    mode = mm_mode or MM_MODE
    key = (mode, n_rep)
    if key in _RUNNER_CACHE:
        return _RUNNER_CACHE[key]

    import jax
    from jax.sharding import Mesh, PartitionSpec
    from jax.experimental.shard_map import shard_map
    from concourse import bass2jax, mybir as _mb

    nc = _get_nc(mode, n_rep)
    bass2jax.install_neuronx_cc_hook()

    pname = nc.partition_id_tensor.name if nc.partition_id_tensor else None
    in_names, out_names, out_avals = [], [], []
    for alloc in nc.m.functions[0].allocations:
        if not isinstance(alloc, _mb.MemoryLocationSet):
            continue
        name = alloc.memorylocations[0].name
        if alloc.kind == "ExternalInput":
            if name != pname:
                in_names.append(name)
        elif alloc.kind == "ExternalOutput":
            out_names.append(name)
            out_avals.append(jax.core.ShapedArray(
                tuple(alloc.tensor_shape), _mb.dt.np(alloc.dtype)))
    n_params = len(in_names)
    all_names = in_names + out_names
    if pname is not None:
        all_names = all_names + [pname]

    def _body(*args):
        operands = list(args)
        if pname is not None:
            operands.append(bass2jax.partition_id_tensor())
        outs = bass2jax._bass_exec_p.bind(
            *operands,
            out_avals=tuple(out_avals),
            in_names=tuple(all_names),
            out_names=tuple(out_names),
            lowering_input_output_aliases=(),
            sim_require_finite=True,
            sim_require_nnan=True,
            nc=nc,
        )
        return tuple(outs)

    devices = jax.devices()[:8]
    mesh = Mesh(np.asarray(devices), ("core",))
    sharded = jax.jit(
        shard_map(_body, mesh=mesh,
                  in_specs=(PartitionSpec("core"),) * (n_params + len(out_names)),
                  out_specs=(PartitionSpec("core"),) * len(out_names),
                  check_rep=False),
        keep_unused=True,
    )

    from jax.sharding import NamedSharding
    zero_outs = [
        jax.device_put(
            np.zeros((8 * a.shape[0], *a.shape[1:]), a.dtype),
            NamedSharding(mesh, PartitionSpec("core")),
        )
        for a in out_avals
    ]

    def run(in_maps):
        concat_in = [
            np.concatenate([np.asarray(m[name]) for m in in_maps], axis=0)
            for name in in_names
        ]
        out_arrs = sharded(*concat_in, *zero_outs)
        return [
            {name: np.asarray(out_arrs[i]).reshape(8, *out_avals[i].shape)[c]
             for i, name in enumerate(out_names)}
            for c in range(8)
        ]

    run.sharded = sharded
    run.in_names = in_names
    run.out_names = out_names
    run.out_avals = out_avals
    run.zero_outs = zero_outs
    run.mesh = mesh
    run.body = _body
    _RUNNER_CACHE[key] = run
    return run


def kernel(x, Wq, bq, Wk, bk, Wv, bv, Wo, bo, **_ignored):
    x = np.asarray(x, dtype=np.float32)
    in_maps = make_in_maps(
        x,
        np.asarray(Wq, np.float32), np.asarray(bq, np.float32),
        np.asarray(Wk, np.float32), np.asarray(bk, np.float32),
        np.asarray(Wv, np.float32), np.asarray(bv, np.float32),
        np.asarray(Wo, np.float32),
    )
    try:
        results = get_runner(MM_MODE)(in_maps)
    except Exception:
        # fallback: stock SPMD runner (slower dispatch, same NEFF)
        from concourse.bass_utils import run_bass_kernel_spmd
        results = run_bass_kernel_spmd(
            _get_nc(MM_MODE), in_maps, core_ids=list(range(8))).results
    return combine_results(results, np.asarray(bo, np.float32))


# revision 39
# speedup vs baseline: 1.3017x; 1.3017x over previous
"""Causal self-attention (D=1024, H=16, S=2048, B=2) on 8 trn2 cores.

Sharding: core i handles batch b = i // 4 and head-group g = i % 4
(4 heads = 256 model dims per group). Each core computes
    y_partial[b,g] = softmax_causal(Q K^T / 8) V  @ Wo[rows of g]
for its 4 heads; the host sums the 4 group partials per batch and adds bo.

Per-core kernel (bf16 matmul operands, fp32 PSUM accumulation), pipelined
over 512-query s-blocks so projections, attention and the output
projection overlap across blocks:
  phase 0: xT[c] <- DMA-transpose of x columns (bf16 XBAR path)
  per s-block sb:
    p1: QT/KT[dc][sb] = (Wq/Wk)^T x^T + b (head pairs packed on
        partitions; bias added on ACT), V tiles for sb's 4 t-chunks
        (+ ones column for the softmax denominator trick)
    p2: per (head-pair dc): scoresT = KT^T QT with causal block skipping,
        exp on ACT (2-plane tiles), triangular diag-chunk mask via gpsimd
        affine_select, PV accumulation -> [65, s] whose row 64 is the
        denominator; reciprocal + DRAM-bounce broadcast + DVE multiply ->
        normalized A^T packed into head-pair tiles AT[dc][sb]
    p3: y[sb] = A Wo with K=128 head-pair accumulation (psum->sbuf on DVE)
"""

import sys

sys.path.insert(0, "/opt/trn_rl_repo")

import ml_dtypes
import numpy as np

import concourse.bass as bass
import concourse.mybir as mybir
import concourse.tile as tile
from concourse import bacc

P = 128
S = 2048
D = 1024
NH = 4                    # heads per core
DH = 64                   # head dim
DPC = NH * DH             # model dims per core = 256
N_CT = D // P             # 8 contraction chunks
N_ST = S // P             # 16 t tiles of 128
N_SB = S // 512           # 4 s blocks of 512
F32 = mybir.dt.float32
BF16 = mybir.dt.bfloat16
SCALE = 1.0 / 8.0         # 1/sqrt(64)

AF = mybir.ActivationFunctionType
ALU = mybir.AluOpType


def build_nc(mm_mode: str = "bf16", stop_after: int = 99,
             skip_norm: bool = False, n_rep: int = 1) -> bass.Bass:
    nc = _build(mm_mode, n_rep)
    if not nc.is_finalized():
        nc.finalize()
    return nc


def _build(mm_mode: str, n_rep: int) -> bass.Bass:
    assert mm_mode == "bf16"
    nc = bacc.Bacc("TRN2", target_bir_lowering=False, debug=False,
                   num_devices=8)

    x_d = nc.dram_tensor("x", [S, D], BF16, kind="ExternalInput")
    wq_d = nc.dram_tensor("wq", [D, DPC], BF16, kind="ExternalInput")
    wk_d = nc.dram_tensor("wk", [D, DPC], BF16, kind="ExternalInput")
    wv_d = nc.dram_tensor("wv", [D, DPC], BF16, kind="ExternalInput")
    wo_d = nc.dram_tensor("wo", [DPC, D], BF16, kind="ExternalInput")
    bq_d = nc.dram_tensor("bq", [DPC], F32, kind="ExternalInput")
    bk_d = nc.dram_tensor("bk", [DPC], F32, kind="ExternalInput")
    bv_d = nc.dram_tensor("bv", [DPC], F32, kind="ExternalInput")
    y_d = nc.dram_tensor("y", [S, D], F32, kind="ExternalOutput")

    with tile.TileContext(nc) as tc:
        with (
            tc.tile_pool(name="const", bufs=1) as const,
            tc.tile_pool(name="xtp", bufs=1) as xtp,
            tc.tile_pool(name="qkv", bufs=1) as qkv,
            tc.tile_pool(name="atp", bufs=1) as atp,
            tc.tile_pool(name="work", bufs=4) as work,
            tc.tile_pool(name="att", bufs=5) as attw,
            tc.tile_pool(name="denp", bufs=4) as denp,
            tc.tile_pool(name="ps", bufs=3, space="PSUM") as psp,
            tc.tile_pool(name="ppv", bufs=2, space="PSUM") as ppv,
        ):
            # ---- HAM warm-up: keep PE busy during the DMA head so the
            # real matmuls start at 2.4 GHz (clock gate released) ----
            wrm = work.tile([P, 512], BF16, tag="wrm")
            nc.vector.memset(wrm, 0.0)
            wps = psp.tile([P, 2, 512], F32, tag="ps")
            for i in range(28):
                nc.tensor.matmul(wps[:, 0, :], wrm[:, 0:P], wrm,
                                 start=True, stop=True)
            ones_s = const.tile([1, DH], BF16)
            nc.vector.memset(ones_s, 1.0)

            # ---- weights / constants (loaded once, ordered by first use) --
            wq_s = const.tile([P, N_CT, DPC], BF16)
            wk_s = const.tile([P, N_CT, DPC], BF16)
            wv_s = const.tile([P, N_CT, DPC], BF16)
            # Wo packed by head pairs: rows 128*dc .. 128*dc+127
            wo_s = const.tile([P, 2, D], BF16)
            nc.sync.dma_start(wq_s, wq_d.rearrange("(o p) d -> p o d", p=P))
            nc.sync.dma_start(wk_s, wk_d.rearrange("(o p) d -> p o d", p=P))
            nc.sync.dma_start(wv_s, wv_d.rearrange("(o p) d -> p o d", p=P))
            nc.sync.dma_start(wo_s, wo_d.rearrange("(dc p) e -> p dc e", p=P))
            bq_s = const.tile([P, 2], F32)
            bk_s = const.tile([P, 2], F32)
            nc.sync.dma_start(bq_s, bq_d.rearrange("(o p) -> p o", p=P))
            nc.sync.dma_start(bk_s, bk_d.rearrange("(o p) -> p o", p=P))
            bv_b = const.tile([P, DPC], F32)
            nc.gpsimd.dma_start(
                out=bv_b, in_=bv_d[:].unsqueeze(0).partition_broadcast(P)
            )

            weights = (wq_s, wk_s, wv_s, wo_s, bq_s, bk_s, bv_b)
            emit_iteration = _make_iteration(nc, tc, const, xtp, qkv, atp,
                                             work, attw, denp, psp, ppv,
                                             ones_s, weights, x_d, wv_d,
                                             wo_d, y_d)
            for rep in range(n_rep):
                emit_iteration(f"r{rep}", rep % 2)

    return nc


def _make_iteration(nc, tc, const, xtp, qkv, atp, work, attw, denp, psp,
                    ppv, ones_s, weights, x_d, wv_d, wo_d, y_d):
    wq_s, wk_s, wv_s, wo_s, bq_s, bk_s, bv_b = weights

    def emit_iteration(sfx, par=0):
            # ---- phase 0: DMA-transpose x into per-chunk xT tiles ----
            # s-block-major so phase-1 groups for early s-blocks can start
            # while later transposes are still on the XBAR. wv/wo loads are
            # interleaved after the transposes their consumers wait behind.
            xT = [xtp.tile([P, S], BF16, tag=f"xt{c}p{par}", name=f"xt{c}{sfx}")
                  for c in range(N_CT)]
            for g in range(N_SB):
                for c in range(N_CT):
                    nc.sync.dma_start_transpose(
                        xT[c][:, g * 512:(g + 1) * 512],
                        x_d[g * 512:(g + 1) * 512, c * P:(c + 1) * P])

            # ---- persistent per-block tiles (fine-grained deps) ----
            # QT/KT: [128 (head-pair d), s] per (dc, sb)
            QTs = [[qkv.tile([P, 512], BF16, tag=f"qt{dc}_{sb}",
                             name=f"qt{dc}_{sb}{sfx}") for sb in range(N_SB)]
                   for dc in range(2)]
            KTs = [[qkv.tile([P, 512], BF16, tag=f"kt{dc}_{sb}",
                             name=f"kt{dc}_{sb}{sfx}") for sb in range(N_SB)]
                   for dc in range(2)]
            # V_aug per t-chunk: [t-part, head, 65], col 64 == 1.0
            vaugs = [qkv.tile([P, NH, DH + 1], BF16, tag=f"va{tt}",
                              name=f"va{tt}{sfx}") for tt in range(N_ST)]
            # normalized A^T packed by head pairs: [128, 512] per (dc, sb)
            ATs = [[atp.tile([P, 512], BF16, tag=f"at{dc}_{sb}",
                             name=f"at{dc}_{sb}{sfx}") for sb in range(N_SB)]
                   for dc in range(2)]

            # ---- emission helpers (software pipelining) ----
            # Per-engine execution is in-order, so program order IS the
            # schedule. Phase-1 groups for block sb+1 and phase-3 stripes
            # for block sb-1 are emitted as "fillers" between attention
            # T-iterations, filling PE bubbles while ACT streams exp.

            # Filler units are (matmul-part, eviction-part) pairs queued
            # separately so the in-order ACT/DVE streams never reach an
            # eviction before its matmul group has finished.

            def emit_qk(dc, sb):
                """Q and K projections for (dc, sb): one 2-plane psum slot;
                eviction (deferred) applies the per-partition biases on ACT."""
                ps = psp.tile([P, 2, 512], F32, tag="ps")
                for i, w_s in enumerate((wq_s, wk_s)):
                    for c in range(N_CT):
                        nc.tensor.matmul(
                            ps[:, i, :],
                            w_s[:, c, dc * P:(dc + 1) * P],
                            xT[c][:, sb * 512:(sb + 1) * 512],
                            start=(c == 0),
                            stop=(c == N_CT - 1),
                        )

                def evict():
                    nc.scalar.activation(
                        QTs[dc][sb], ps[:, 0, :], AF.Identity,
                        bias=bq_s[:, dc:dc + 1],
                    )
                    nc.scalar.activation(
                        KTs[dc][sb], ps[:, 1, :], AF.Identity,
                        bias=bk_s[:, dc:dc + 1],
                    )
                return evict

            def emit_v(sb, ti):
                """V projections for t-tiles (4sb+2ti, 4sb+2ti+1): one
                2-plane psum slot; deferred DVE bias adds."""
                ps = psp.tile([P, 2, 512], F32, tag="ps")
                for i in range(2):
                    tt = 4 * sb + 2 * ti + i
                    pvs = ps[:, i, 0:DPC]
                    for c in range(N_CT):
                        nc.tensor.matmul(
                            pvs,
                            xT[c][:, tt * P:(tt + 1) * P],
                            wv_s[:, c, :],
                            start=(c == 0),
                            stop=(c == N_CT - 1),
                        )

                def evict():
                    for i in range(2):
                        tt = 4 * sb + 2 * ti + i
                        nc.vector.memset(vaugs[tt][:, :, DH:DH + 1], 1.0)
                        nc.vector.tensor_add(
                            vaugs[tt][:, :, 0:DH],
                            ps[:, i, 0:DPC].rearrange("p (h u) -> p h u", h=NH),
                            bv_b.rearrange("p (h u) -> p h u", h=NH),
                        )
                return evict

            def emit_p3(sb, stl):
                """Output-projection stripe: both half-blocks of one 128-row
                stripe share a 2-plane psum slot; deferred DVE eviction."""
                ps = psp.tile([P, 2, 512], F32, tag="ps")
                for eb in range(2):
                    for dc in range(2):
                        nc.tensor.matmul(
                            ps[:, eb, :],
                            ATs[dc][sb][:, stl * P:(stl + 1) * P],
                            wo_s[:, dc, eb * 512:(eb + 1) * 512],
                            start=(dc == 0),
                            stop=(dc == 1),
                        )

                def evict():
                    ys = work.tile([P, D], F32, tag="work")
                    nc.vector.tensor_copy(
                        ys, ps.rearrange("p a b -> p (a b)"))
                    st = 4 * sb + stl
                    nc.sync.dma_start(y_d[st * P:(st + 1) * P, :], ys)
                return evict

            def emit_normalize_pair(dc, sb, pvs2):
                """Evict both PV psums to SBUF first (frees the ppv slots
                fast), then reciprocal + PE ones-matmul broadcast + DVE
                multiply per head."""
                # e=1 evicts on ACT (free right after the last exp), e=0 on
                # DVE — both PV slots free in parallel, recips start sooner
                pvcs = [denp.tile([DH + 1, 512], F32, tag=f"pvc{e}",
                                  name=f"pvc{e}{sfx}")
                        for e in range(2)]
                nc.scalar.copy(pvcs[1], pvs2[1])
                nc.vector.tensor_copy(pvcs[0], pvs2[0])
                for e in (1, 0):  # e=1 first: its extra AT DMA gains latency
                    pvc = pvcs[e]
                    rden = denp.tile([1, 512], BF16, tag="rden",
                                     name=f"rden{sfx}")
                    with nc.allow_low_precision(
                            reason="softmax denom recip, 2e-2 tol"):
                        nc.vector.reciprocal(out=rden, in_=pvc[DH:DH + 1, :])
                    # broadcast recip down 64 partitions: [1,DH].T @ [1,512]
                    rb = ppv.tile([DH, 512], F32, tag="pv",
                                  name=f"rb{dc}{sb}{e}{sfx}")
                    nc.tensor.matmul(rb, ones_s, rden, start=True, stop=True)
                    if e == 0:
                        nc.vector.tensor_mul(
                            ATs[dc][sb][0:DH, :], pvc[0:DH, :], rb)
                    else:
                        att = attw.tile([DH, 512], BF16, tag="att")
                        nc.vector.tensor_mul(att, pvc[0:DH, :], rb)
                        nc.sync.dma_start(ATs[dc][sb][DH:P, :], att)

            # filler units, queued in dependency-deadline order; a unit is
            # (fn, pace): fn may return an eviction closure, re-queued as
            # the next unit so it runs ~pace T-iterations after the matmuls
            from collections import deque
            fillers = deque()

            def pop_filler(T, next_ok):
                fn, pace = fillers.popleft()
                nxt = fn()
                if nxt is not None:
                    fillers.appendleft((nxt, 1))
                next_ok[0] = T + pace

            # head: everything phase 2 of (dc=0, sb=0) needs, plus the
            # next block's first QK group (the DMA head has PE slack)
            emit_qk(0, 0)()
            emit_v(0, 0)()
            emit_v(0, 1)()
            emit_qk(0, 1)()

            for sb in range(N_SB):
                t_cnt = 4 * sb + 4
                for dc in range(2):
                    if dc == 0:
                        # queue next block's phase 1 at each period start;
                        # phase-3 stripes go late (sb2: block 0; sb3:
                        # blocks 1+2) where the long exp chains leave the
                        # most PE slack
                        if sb == 0:
                            fillers.append((lambda: emit_qk(1, 0), 2))
                            fillers.append((lambda: emit_v(1, 0), 2))
                            fillers.append((lambda: emit_v(1, 1), 2))
                            fillers.append((lambda: emit_qk(1, 1), 2))
                        elif sb + 1 < N_SB:
                            fillers.append(
                                (lambda s=sb + 1: emit_qk(0, s), 2))
                            fillers.append(
                                (lambda s=sb + 1: emit_v(s, 0), 2))
                            fillers.append(
                                (lambda s=sb + 1: emit_v(s, 1), 2))
                            fillers.append(
                                (lambda s=sb + 1: emit_qk(1, s), 2))
                        for s in ([0] if sb == 2 else [1, 2] if sb == 3
                                  else []):
                            for stl in range(4):
                                fillers.append(
                                    (lambda s=s, i=stl: emit_p3(s, i), 2))
                    pvs2 = [ppv.tile([DH + 1, 512], F32, tag="pv",
                                     name=f"pv{dc}_{sb}_{e}{sfx}")
                            for e in range(2)]

                    def emit_pv(T, ex, ms):
                        for e in range(2):
                            h = 2 * dc + e
                            nc.tensor.matmul(
                                pvs2[e][:, ms:512],
                                vaugs[T][:, h, :],
                                ex[:, e, ms:512],
                                start=(T == 0),
                                stop=(T == t_cnt - 1),
                            )

                    # PV emission lags scores/exp by 2 iterations so the
                    # half's first PV (which waits on the previous half's
                    # ppv slots) never blocks the in-order PE stream
                    pv_q = deque()
                    next_ok = [1]
                    for T in range(t_cnt):
                        k = T - 4 * sb
                        ms = 128 * k if k > 0 else 0
                        sc = psp.tile([P, 2, 512], F32, tag="ps")
                        ex = attw.tile([P, 2, 512], BF16, tag="ex")
                        for e in range(2):  # even/odd head of the pair
                            off = DH * e
                            nc.tensor.matmul(
                                sc[:, e, ms:512],
                                KTs[dc][T // 4][off:off + DH,
                                                (T % 4) * P:(T % 4 + 1) * P],
                                QTs[dc][sb][off:off + DH, ms:512],
                                start=True,
                                stop=True,
                            )
                        nc.scalar.activation(
                            ex[:, :, ms:512], sc[:, :, ms:512],
                            AF.Exp, scale=SCALE,
                        )
                        if k >= 0:  # triangular mask on diagonal chunks
                            nc.gpsimd.affine_select(
                                out=ex[:, :, ms:ms + P],
                                in_=ex[:, :, ms:ms + P],
                                compare_op=ALU.is_ge,
                                fill=0.0,
                                base=0,
                                pattern=[[0, 2], [1, P]],
                                channel_multiplier=-1,
                            )
                        pv_q.append((T, ex, ms))
                        if len(pv_q) > 2:
                            emit_pv(*pv_q.popleft())
                        if fillers and T >= next_ok[0]:
                            pop_filler(T, next_ok)
                    while pv_q:
                        emit_pv(*pv_q.popleft())
                    emit_normalize_pair(dc, sb, pvs2)
                # tail: drain the queue, then the last block's phase 3
                if sb == N_SB - 1:
                    while fillers:
                        fn, _ = fillers.popleft()
                        nxt = fn()
                        if nxt is not None:
                            fillers.appendleft((nxt, 1))
                    for stl in range(4):
                        emit_p3(sb, stl)()

    return emit_iteration


_NC_CACHE = {}


def _get_nc(mm_mode="bf16", n_rep=1):
    key = (mm_mode, n_rep)
    if key not in _NC_CACHE:
        _NC_CACHE[key] = build_nc(mm_mode=mm_mode, n_rep=n_rep)
    return _NC_CACHE[key]


MM_MODE = "bf16"


def make_in_maps(x, Wq, bq, Wk, bk, Wv, bv, Wo, mm_mode=None):
    """Per-core input dicts: core i -> (batch i//4, head-group i%4)."""
    bf = ml_dtypes.bfloat16
    in_maps = []
    for core in range(8):
        b, g = core // 4, core % 4
        sl = slice(g * DPC, (g + 1) * DPC)
        in_maps.append({
            "x": np.ascontiguousarray(x[b]).astype(bf),
            "wq": np.ascontiguousarray(Wq[:, sl]).astype(bf),
            "wk": np.ascontiguousarray(Wk[:, sl]).astype(bf),
            "wv": np.ascontiguousarray(Wv[:, sl]).astype(bf),
            "wo": np.ascontiguousarray(Wo[sl, :]).astype(bf),
            "bq": np.ascontiguousarray(bq[sl]).astype(np.float32),
            "bk": np.ascontiguousarray(bk[sl]).astype(np.float32),
            "bv": np.ascontiguousarray(bv[sl]).astype(np.float32),
        })
    return in_maps


def combine_results(results, bo):
    out = np.zeros((2, S, D), dtype=np.float32)
    for core in range(8):
        out[core // 4] += results[core]["y"]
    out += bo.astype(np.float32)
    return out


_RUNNER_CACHE = {}


def get_runner(mm_mode=None, n_rep=1):
    """Build (once) a jitted 8-core runner; returns fn(in_maps) -> results."""
    mode = mm_mode or MM_MODE
    key = (mode, n_rep)
    if key in _RUNNER_CACHE:
        return _RUNNER_CACHE[key]

    import jax
    from jax.sharding import Mesh, PartitionSpec
    from jax.experimental.shard_map import shard_map
    from concourse import bass2jax, mybir as _mb

    nc = _get_nc(mode, n_rep)
    bass2jax.install_neuronx_cc_hook()

    pname = nc.partition_id_tensor.name if nc.partition_id_tensor else None
    in_names, out_names, out_avals = [], [], []
    for alloc in nc.m.functions[0].allocations:
        if not isinstance(alloc, _mb.MemoryLocationSet):
            continue
        name = alloc.memorylocations[0].name
        if alloc.kind == "ExternalInput":
            if name != pname:
                in_names.append(name)
        elif alloc.kind == "ExternalOutput":
            out_names.append(name)
            out_avals.append(jax.core.ShapedArray(
                tuple(alloc.tensor_shape), _mb.dt.np(alloc.dtype)))
    n_params = len(in_names)
    all_names = in_names + out_names
    if pname is not None:
        all_names = all_names + [pname]

    def _body(*args):
        operands = list(args)
        if pname is not None:
            operands.append(bass2jax.partition_id_tensor())
        outs = bass2jax._bass_exec_p.bind(
            *operands,
            out_avals=tuple(out_avals),
            in_names=tuple(all_names),
            out_names=tuple(out_names),
            lowering_input_output_aliases=(),
            sim_require_finite=True,
            sim_require_nnan=True,
            nc=nc,
        )
        return tuple(outs)

    devices = jax.devices()[:8]
    mesh = Mesh(np.asarray(devices), ("core",))
    sharded = jax.jit(
        shard_map(_body, mesh=mesh,
                  in_specs=(PartitionSpec("core"),) * (n_params + len(out_names)),
                  out_specs=(PartitionSpec("core"),) * len(out_names),
                  check_rep=False),
        keep_unused=True,
    )

    from jax.sharding import NamedSharding
    zero_outs = [
        jax.device_put(
            np.zeros((8 * a.shape[0], *a.shape[1:]), a.dtype),
            NamedSharding(mesh, PartitionSpec("core")),
        )
        for a in out_avals
    ]

    def run(in_maps):
        concat_in = [
            np.concatenate([np.asarray(m[name]) for m in in_maps], axis=0)
            for name in in_names
        ]
        out_arrs = sharded(*concat_in, *zero_outs)
        return [
            {name: np.asarray(out_arrs[i]).reshape(8, *out_avals[i].shape)[c]
             for i, name in enumerate(out_names)}
            for c in range(8)
        ]

    run.sharded = sharded
    run.in_names = in_names
    run.out_names = out_names
    run.out_avals = out_avals
    run.zero_outs = zero_outs
    run.mesh = mesh
    run.body = _body
    _RUNNER_CACHE[key] = run
    return run


def kernel(x, Wq, bq, Wk, bk, Wv, bv, Wo, bo, **_ignored):
    x = np.asarray(x, dtype=np.float32)
    in_maps = make_in_maps(
        x,
        np.asarray(Wq, np.float32), np.asarray(bq, np.float32),
        np.asarray(Wk, np.float32), np.asarray(bk, np.float32),
        np.asarray(Wv, np.float32), np.asarray(bv, np.float32),
        np.asarray(Wo, np.float32),
    )
    try:
        results = get_runner(MM_MODE)(in_maps)
    except Exception:
        # fallback: stock SPMD runner (slower dispatch, same NEFF)
        from concourse.bass_utils import run_bass_kernel_spmd
        results = run_bass_kernel_spmd(
            _get_nc(MM_MODE), in_maps, core_ids=list(range(8))).results
    return combine_results(results, np.asarray(bo, np.float32))
